# revision 4
# baseline (speedup 1.0000x reference)
"""Cost-volume construction kernel for Trainium2 (8 NeuronCores).

Reference computation (N=1, C=32, H=128, W=240, max_disparity=192, D4=48):
  out[0, c,     i, h, w] = left[0, c, h, w]      if w >= i else 0   (c in [0,32))
  out[0, 32+c,  i, h, w] = right[0, c, h, w-i]   if w >= i else 0

Pure data movement (377 MB output from 8 MB of inputs) -> DMA-write bound.
The cost model serializes all DMA transfers on the per-core DMA-engine pool
at 360 B/ns, so runtime ~= bytes moved / 360 GB/s + fixed edges.

Sharding: H is split 8 ways (16 rows per core) so every core runs the exact
same program on its H-slice -- no core-dependent constants needed for SPMD.

Per-core kernel: the output buffer is donated from jnp.zeros, so the zero
prefix of each disparity slice (cols [0:i)) is never written. Each disparity
is two direct DRAM->DRAM windowed copies (no SBUF staging, no input loads,
no inter-engine sync):
  out[0:C,  i, :, i:] <- left [:, :, i:]
  out[C:2C, i, :, i:] <- right[:, :, :W-i]
Both sides are 3-dim access patterns with >=772B contiguous rows, so every
DMA runs at the full modeled descriptor rate. vs. writing all bytes from
SBUF this saves the 9.8% zero-region writes plus the 1 MB input load.
"""

import numpy as np

C = 32
H = 128
W = 240
D4 = 48
N_CORES = 8
HC = H // N_CORES  # 16 rows per core

_CACHE = {}


def _build_bass():
    import concourse.bass as bass
    import concourse.mybir as mybir

    f32 = mybir.dt.float32
    nc = bass.Bass(trn_type="TRN2")
    L = nc.dram_tensor("left", (C, HC, W), f32, kind="ExternalInput")
    R = nc.dram_tensor("right", (C, HC, W), f32, kind="ExternalInput")
    O = nc.dram_tensor("out", (2 * C, D4, HC, W), f32, kind="ExternalOutput")

    with (
        nc.Block() as block,
        nc.semaphore("st") as st,
    ):
        @block.sync
        def _(sync):
            for i in range(D4):
                sync.dma_start(out=O[0:C, i, :, i:], in_=L[:, :, i:]).then_inc(st, 16)
                sync.dma_start(out=O[C:, i, :, i:], in_=R[:, :, 0 : W - i]).then_inc(st, 16)
            sync.wait_ge(st, 16 * 2 * D4)

    return nc


def _get_nc():
    if "nc" not in _CACHE:
        _CACHE["nc"] = _build_bass()
    return _CACHE["nc"]


def _get_exec():
    """Build and cache the jitted SPMD executable (with output donation) and
    a device-side zero-buffer maker, so repeat kernel() calls only pay
    input upload + execution + output download. The donated zero buffer is
    load-bearing: unwritten output regions must read back as zeros."""
    if "exec" in _CACHE:
        return _CACHE["exec"]

    import jax
    import jax.numpy as jnp
    from jax.sharding import Mesh, NamedSharding, PartitionSpec
    from jax.experimental.shard_map import shard_map
    import concourse.mybir as mybir
    from concourse import bass2jax

    nc = _get_nc()
    bass2jax.install_neuronx_cc_hook()
    partition_name = nc.partition_id_tensor.name if nc.partition_id_tensor else None

    in_names, out_names, out_avals = [], [], []
    for alloc in nc.m.functions[0].allocations:
        if not isinstance(alloc, mybir.MemoryLocationSet):
            continue
        name = alloc.memorylocations[0].name
        if alloc.kind == "ExternalInput":
            if name != partition_name:
                in_names.append(name)
        elif alloc.kind == "ExternalOutput":
            out_names.append(name)
            out_avals.append(
                jax.core.ShapedArray(tuple(alloc.tensor_shape), mybir.dt.np(alloc.dtype))
            )
    n_params = len(in_names)
    all_names = list(in_names) + out_names
    if partition_name is not None:
        all_names.append(partition_name)

    def _body(*args):
        operands = list(args)
        if partition_name is not None:
            operands.append(bass2jax.partition_id_tensor())
        outs = bass2jax._bass_exec_p.bind(
            *operands,
            out_avals=tuple(out_avals),
            in_names=tuple(all_names),
            out_names=tuple(out_names),
            lowering_input_output_aliases=(),
            sim_require_finite=True,
            sim_require_nnan=True,
            nc=nc,
        )
        return tuple(outs)

    devices = jax.devices()[:N_CORES]
    mesh = Mesh(np.asarray(devices), ("core",))
    spec = PartitionSpec("core")
    n_outs = len(out_names)
    donate = tuple(range(n_params, n_params + n_outs))
    fn = jax.jit(
        shard_map(
            _body,
            mesh=mesh,
            in_specs=(spec,) * (n_params + n_outs),
            out_specs=(spec,) * n_outs,
            check_rep=False,
        ),
        donate_argnums=donate,
        keep_unused=True,
    )

    sharding = NamedSharding(mesh, spec)
    zero_makers = [
        jax.jit(
            lambda aval=aval: jnp.zeros((N_CORES * aval.shape[0], *aval.shape[1:]), aval.dtype),
            out_shardings=sharding,
        )
        for aval in out_avals
    ]
    _CACHE["exec"] = (fn, in_names, zero_makers, sharding)
    return _CACHE["exec"]


def kernel(left_feature, right_feature, max_disparity=192):
    import jax

    assert int(max_disparity) == D4 * 4
    lf = np.ascontiguousarray(np.asarray(left_feature, dtype=np.float32)).reshape(C, H, W)
    rf = np.ascontiguousarray(np.asarray(right_feature, dtype=np.float32)).reshape(C, H, W)

    fn, in_names, zero_makers, sharding = _get_exec()
    # global (concat-over-cores) input arrays; core k's shard is its H-slice
    host_in = {
        "left": lf.transpose(1, 0, 2).reshape(N_CORES, HC, C, W).transpose(0, 2, 1, 3).reshape(N_CORES * C, HC, W),
        "right": rf.transpose(1, 0, 2).reshape(N_CORES, HC, C, W).transpose(0, 2, 1, 3).reshape(N_CORES * C, HC, W),
    }

    last_exc = None
    for attempt in range(3):
        args = []
        try:
            args = [jax.device_put(np.ascontiguousarray(host_in[nm]), sharding) for nm in in_names]
            args += [zm() for zm in zero_makers]
            (out_g,) = fn(*args)
            out = np.asarray(out_g)  # (8*64, 48, 16, 240)
            out_g.delete()
            break
        except Exception as exc:  # transient axon/NRT hiccups: retry
            last_exc = exc
            import time

            time.sleep(5 * (attempt + 1))
    else:
        raise last_exc
    # free device buffers promptly so the terminal stays light for the next
    # session attach (stale multi-hundred-MB buffers slow it down a lot)
    for a in args:
        try:
            if not a.is_deleted():
                a.delete()
        except Exception:
            pass
    # core k owns H rows [16k, 16k+16): reassemble to (64, 48, 128, 240)
    full = out.reshape(N_CORES, 2 * C, D4, HC, W).transpose(1, 2, 0, 3, 4).reshape(2 * C, D4, H, W)
    return np.ascontiguousarray(full).reshape(1, 2 * C, D4, H, W)
